# revision 4
# baseline (speedup 1.0000x reference)
"""KAN layer (B-spline spline + SiLU base) Trainium2 Bass kernel.

Strategy (data-parallel over batch N across 8 cores, params replicated):

The cubic B-spline basis on a per-row-uniform grid is a linear combination
of truncated cubes:  B_b(x) = M(y-b),  y = (x-g0)/h,
  M(s) = (1/6) * sum_k (-1)^k C(4,k) (s-k)+^3.
That linear map (and w_sp, C) folds into the matmul weights on the host, so
the device only computes, per feature-group of 8 input features:
  1. PE:  yt[(ib,t), n] = alpha_i * x[n,i] + (beta_i - t)   (broadcast matmul,
          with 8 extra rows carrying raw x for the SiLU path)
  2. ACT: relu(yt), square(yt)  ->  DVE: cube = relu*square = (y-t)+^3
          ACT: silu rows
  3. PE:  post_act[n, (i,j)] = cube^T @ Wcc   (block-diagonal weights,
          includes the w_base*silu term via the silu rows)
          out[n, j] accumulated with a second matmul against dense weights.
Partition packing per group: rows ib*12+s (s=0..10 cube slots, s=11 pad),
rows 96..103 silu, rows 104..127 zero-pad.  Slots t>=11 are dropped: for
x in [-1,1) they are exactly zero ((y-t)+^3 with y<11).
"""

import math

import numpy as np

N_TOT = 4096
N_IN = 64
N_OUT = 64
N_BASES = 11
N_KNOTS = 15
N_CORES = 8
NSH = N_TOT // N_CORES          # 512 rows per core
NCHUNKS = NSH // 128            # 4 partition chunks of the batch shard
NGRP = 8                        # feature groups of 8
GP = 8                          # features per group
SLOT = 12                       # cube slots per feature (11 used + pad)
GROWS = 128                     # rows per group tile (8*15 + 8 silu)

_CACHE = {}


def _build_module():
    import concourse.bass as bass  # noqa: F401
    import concourse.mybir as mybir
    from concourse import bacc
    from concourse.tile import TileContext

    dt = mybir.dt.float32
    nc = bacc.Bacc(target_bir_lowering=False)

    xo = nc.dram_tensor("xo", [NGRP * 9, NSH], dt, kind="ExternalInput")
    S = nc.dram_tensor("S", [NGRP * 9, GROWS], dt, kind="ExternalInput")
    Wcc = nc.dram_tensor("Wcc", [NGRP * GROWS, 512], dt, kind="ExternalInput")
    WccD = nc.dram_tensor("WccD", [NGRP * GROWS, N_OUT], dt, kind="ExternalInput")
    pa = nc.dram_tensor("pa", [NSH, N_IN * N_OUT], dt, kind="ExternalOutput")
    outp = nc.dram_tensor("outp", [NSH, N_OUT], dt, kind="ExternalOutput")

    AF = mybir.ActivationFunctionType

    with TileContext(nc) as tc:
        with tc.tile_pool(name="consts", bufs=1) as cpool, \
             tc.tile_pool(name="cubes", bufs=1) as cubepool, \
             tc.tile_pool(name="work", bufs=3) as wpool, \
             tc.tile_pool(name="pas", bufs=6) as papool, \
             tc.tile_pool(name="yt", bufs=2, space="PSUM") as ytpool, \
             tc.tile_pool(name="paps", bufs=4, space="PSUM") as papspool, \
             tc.tile_pool(name="outps", bufs=2, space="PSUM") as outpspool:

            S_sb = []
            xo_sb = []
            for g in range(NGRP):
                sg = cpool.tile([9, GROWS], dt, tag=f"S{g}")
                nc.sync.dma_start(out=sg[:], in_=S[9 * g:9 * g + 9, :])
                S_sb.append(sg)
                xg = cpool.tile([9, NSH], dt, tag=f"xo{g}")
                nc.sync.dma_start(out=xg[:], in_=xo[9 * g:9 * g + 9, :])
                xo_sb.append(xg)

            wcc_sb = []
            wccd_sb = []
            for g in range(NGRP):
                w = cpool.tile([GROWS, 512], dt, tag=f"wcc{g}")
                nc.sync.dma_start(out=w[:], in_=Wcc[g * GROWS:(g + 1) * GROWS, :])
                wcc_sb.append(w)
                wd = cpool.tile([GROWS, N_OUT], dt, tag=f"wccd{g}")
                nc.sync.dma_start(out=wd[:], in_=WccD[g * GROWS:(g + 1) * GROWS, :])
                wccd_sb.append(wd)

            # ---- Phase A: build cube tiles (one per feature group) ----
            cubes = []
            for g in range(NGRP):
                yt = ytpool.tile([GROWS, NSH], dt, tag="yt")
                nc.tensor.matmul(out=yt[:], lhsT=S_sb[g][:],
                                 rhs=xo_sb[g][:],
                                 start=True, stop=True)
                q = wpool.tile([GROWS, NSH], dt, tag="q")
                q2 = wpool.tile([GROWS, NSH], dt, tag="q2")
                cube = cubepool.tile([GROWS, NSH], dt, tag=f"cube{g}")
                nc.scalar.activation(q[:, :], yt[:, :], AF.Relu)
                nc.scalar.activation(q2[:, :], yt[:, :], AF.Square)
                nc.vector.tensor_mul(out=cube[:, :], in0=q[:, :], in1=q2[:, :])
                nc.scalar.activation(cube[96:104, :], yt[96:104, :], AF.Silu)
                cubes.append(cube)

            # ---- Phase B: post_act + out matmuls per (n-chunk, group) ----
            for c in range(NCHUNKS):
                nsl = slice(c * 128, (c + 1) * 128)
                out_ps = outpspool.tile([128, N_OUT], dt, tag="outps")
                for g in range(NGRP):
                    pa_ps = papspool.tile([128, 512], dt, tag="paps")
                    nc.tensor.matmul(out=pa_ps[:], lhsT=cubes[g][:, nsl],
                                     rhs=wcc_sb[g][:], start=True, stop=True)
                    nc.tensor.matmul(out=out_ps[:], lhsT=cubes[g][:, nsl],
                                     rhs=wccd_sb[g][:], start=(g == 0),
                                     stop=(g == NGRP - 1), skip_group_check=True)
                    pa_sb = papool.tile([128, 512], dt, tag="pa")
                    if g % 4 == 3:
                        nc.scalar.copy(pa_sb[:], pa_ps[:])
                    else:
                        nc.vector.tensor_copy(pa_sb[:], pa_ps[:])
                    nc.sync.dma_start(out=pa[nsl, g * 512:(g + 1) * 512],
                                      in_=pa_sb[:])
                out_sb = papool.tile([128, N_OUT], dt, tag="outsb")
                nc.vector.tensor_copy(out_sb[:], out_ps[:])
                nc.sync.dma_start(out=outp[nsl, :], in_=out_sb[:])

    nc.compile()
    return nc


def _host_prep(x, grid, C, w_base, w_sp):
    f32 = np.float32
    x = np.asarray(x, f32)
    grid = np.asarray(grid, f32)
    C = np.asarray(C, f32)
    w_base = np.asarray(w_base, f32)
    w_sp = np.asarray(w_sp, f32)

    h = grid[:, 1] - grid[:, 0]
    g0 = grid[:, 0]
    alpha = (1.0 / h).astype(f32)

    # T4[t,b]: truncated-cube -> B-spline band matrix (taps t>=11 dropped:
    # exactly zero for x below grid[:, 11])
    T4 = np.zeros((SLOT, N_BASES), f32)
    for b in range(N_BASES):
        for k in range(5):
            if b + k < 11:
                T4[b + k, b] = ((-1.0) ** k) * math.comb(4, k) / 6.0
    # Wfull[i,t,j] = w_sp[i,j] * sum_b T4[t,b] C[i,j,b]   (C is [i, j, b])
    Wfull = np.einsum("tb,ijb->itj", T4, C).astype(f32) * w_sp[:, None, :]

    S = np.zeros((NGRP * 9, GROWS), f32)
    t_idx = np.arange(11, dtype=f32)
    for g in range(NGRP):
        for ib in range(GP):
            i = g * GP + ib
            S[9 * g + ib, ib * SLOT:ib * SLOT + 11] = alpha[i]
            S[9 * g + 8, ib * SLOT:ib * SLOT + 11] = -g0[i] * alpha[i] - t_idx
            S[9 * g + ib, 96 + ib] = 1.0
    Wcc = np.zeros((NGRP * GROWS, 512), f32)
    WccD = np.zeros((NGRP * GROWS, N_OUT), f32)
    for g in range(NGRP):
        for ib in range(GP):
            i = g * GP + ib
            r0 = g * GROWS + ib * SLOT
            Wcc[r0:r0 + SLOT, ib * N_OUT:(ib + 1) * N_OUT] = Wfull[i]
            Wcc[g * GROWS + 96 + ib, ib * N_OUT:(ib + 1) * N_OUT] = w_base[i]
            WccD[r0:r0 + SLOT, :] = Wfull[i]
            WccD[g * GROWS + 96 + ib, :] = w_base[i]

    xT = np.ascontiguousarray(x.T)  # (n_in, N)
    in_maps = []
    for cidx in range(N_CORES):
        xs = xT[:, cidx * NSH:(cidx + 1) * NSH]
        xo = np.empty((NGRP * 9, NSH), f32)
        for g in range(NGRP):
            xo[9 * g:9 * g + 8] = xs[g * GP:(g + 1) * GP]
            xo[9 * g + 8] = 1.0
        in_maps.append({"xo": np.ascontiguousarray(xo), "S": S,
                        "Wcc": Wcc, "WccD": WccD})
    return in_maps


def _get_nc():
    if "nc" not in _CACHE:
        _CACHE["nc"] = _build_module()
    return _CACHE["nc"]


def run_on_device(in_maps):
    from concourse.bass_utils import run_bass_kernel_spmd
    return run_bass_kernel_spmd(_get_nc(), in_maps,
                                core_ids=list(range(N_CORES)))


def kernel(x, grid, C, w_base, w_sp):
    in_maps = _host_prep(x, grid, C, w_base, w_sp)
    res = run_on_device(in_maps)
    pa = np.concatenate([res.results[c]["pa"] for c in range(N_CORES)], axis=0)
    out = np.concatenate([res.results[c]["outp"] for c in range(N_CORES)],
                         axis=0)
    post_act = pa.reshape(N_TOT, N_IN, N_OUT)
    return (out.astype(np.float32), post_act.astype(np.float32))
